# revision 42
# baseline (speedup 1.0000x reference)
"""Trainium2 Bass kernel for BubbleformerAttentionBlock.

Sharding: 8 cores = 2 batch (B) x 4 pixel-row blocks (8 rows of 32 each).
Fast path (qnorm/knorm affine trivial, attn_scale_factor==1, zero biases):
single x load, instance-norm1 (stats AllReduce'd across the 4 cores of the
same batch), token-major qkv matmul with LN folded into centered-q /
scaled-k (softmax scores are invariant to k centering once q is centered),
per-8px-group masked attention over N=16 tokens, norm2 statistics
accumulated on the PE via selection matmuls during attention, output
projection overlapped with the norm2 apply.
"""
import sys

for _p in ("/opt/trn_rl_repo", "/opt/trn_rl_repo/concourse"):
    if _p not in sys.path:
        sys.path.insert(0, _p)

import numpy as np
import ml_dtypes

B, N, EMB, HH, WW, HEADS, HD = 2, 16, 768, 32, 32, 12, 64
EPS = 1e-5
PX = 256            # pixels per core (8 rows x 32)
NG = PX // 8        # 32 token-groups of 8 pixels
CB = EMB // 128     # 6 channel blocks
CO = 3 * EMB        # 2304 qkv output channels
SCALE = float(HD) ** -0.5
NCORES = 8

bf16 = ml_dtypes.bfloat16

_prog_cache = {}


def _pin_act_tables():
    import concourse.bacc as bacc
    # All ACT functions used here (Exp, Ln, Square, Identity, Copy) live in
    # the natural_log_exp_and_others table set; blank out the other sets
    # (keeping their ids) so one table load covers the whole kernel.
    if not getattr(bacc, "_act_tables_pinned", False):
        _orig_gat = bacc.get_activation_tables

        def _pinned(arch):
            t = _orig_gat(arch)
            return {k: (v if k == "natural_log_exp_and_others" else type(v)())
                    for k, v in t.items()}

        bacc.get_activation_tables = _pinned
        bacc._act_tables_pinned = True


def _build_fast(for_sim=False, ystat_psum=True):
    import concourse.bacc as bacc
    import concourse.mybir as mybir
    import concourse.tile as tile

    _pin_act_tables()

    dt = mybir.dt
    AF = mybir.ActivationFunctionType
    AL = mybir.AluOpType

    nc = bacc.Bacc("TRN2", target_bir_lowering=False, debug=False, num_devices=NCORES)

    def din(name, shape, d=dt.float32):
        return nc.dram_tensor(name, list(shape), d, kind="ExternalInput").ap()

    xs = din("xs", (N, EMB, PX))
    wq = din("wq", (EMB, CO), dt.bfloat16)        # W_qkv^T, head-major q|k|v
    w2 = din("w2", (EMB, EMB), dt.bfloat16)       # W_out^T
    n1w = din("n1w", (EMB,))
    n1b = din("n1b", (EMB,))
    n2w = din("n2w", (EMB,))
    n2b = din("n2b", (EMB,))
    ident = din("ident", (128, 128), dt.bfloat16)
    mask4 = din("mask4", (128, 512), dt.bfloat16)
    sel16 = din("sel16", (128, 16), dt.bfloat16)
    mk9 = din("mk9", (9, 256), dt.bfloat16)
    out = nc.dram_tensor("out", [N, EMB, PX], dt.float32, kind="ExternalOutput").ap()

    with tile.TileContext(nc) as tc:
        with tc.tile_pool(name="const", bufs=1) as cp, \
             tc.tile_pool(name="wts", bufs=1) as wp, \
             tc.tile_pool(name="dram", bufs=1, space="DRAM") as dp, \
             tc.tile_pool(name="stats", bufs=2) as stp:
            xnp_cm = tc.tile_pool(name="xnyn", bufs=1)
            xnp = xnp_cm.__enter__()

            # ---- constants ----
            id_sb = cp.tile([128, 128], dt.bfloat16)
            nc.sync.dma_start(id_sb[:], ident[:])
            sel_sb = cp.tile([128, 16], dt.bfloat16)
            nc.sync.dma_start(sel_sb[:], sel16[:])
            mk9_sb = cp.tile([9, 256], dt.bfloat16)
            nc.sync.dma_start(mk9_sb[:], mk9[:])
            eps_c = cp.tile([128, 1], dt.float32)
            nc.vector.memset(eps_c[:], EPS)
            g1c = cp.tile([128, CB], dt.float32)
            nc.sync.dma_start(g1c[:], n1w.rearrange("(cb c) -> c cb", c=128))
            b1c = cp.tile([128, CB], dt.float32)
            nc.sync.dma_start(b1c[:], n1b.rearrange("(cb c) -> c cb", c=128))
            g2c = cp.tile([128, CB], dt.float32)
            nc.sync.dma_start(g2c[:], n2w.rearrange("(cb c) -> c cb", c=128))
            b2c = cp.tile([128, CB], dt.float32)
            nc.sync.dma_start(b2c[:], n2b.rearrange("(cb c) -> c cb", c=128))

            def norm_coeffs(statsr, gc, bc, inv_count, prefix):
                """statsr (128, 2, CB, N) summed stats -> alpha,beta (128, CB, N)."""
                mue2 = stp.tile([128, 2, CB, N], dt.float32, tag=prefix + "mu", name=prefix + "mu")
                nc.vector.tensor_scalar(mue2[:], statsr[:], inv_count, None, AL.mult)
                mu = mue2[:, 0]
                e2 = mue2[:, 1]
                msq = stp.tile([128, CB, N], dt.float32, tag=prefix + "msq", name=prefix + "msq")
                nc.scalar.activation(msq[:], mu, AF.Square)
                var = stp.tile([128, CB, N], dt.float32, tag=prefix + "var", name=prefix + "var")
                nc.vector.tensor_sub(var[:], e2, msq[:])
                # rstd = exp(-0.5*ln(var+eps)) -- keeps ACT in the exp/ln table set
                lv = stp.tile([128, CB, N], dt.float32, tag=prefix + "lv", name=prefix + "lv")
                nc.scalar.activation(lv[:], var[:], AF.Ln, bias=eps_c[:])
                rstd = stp.tile([128, CB, N], dt.float32, tag=prefix + "rstd", name=prefix + "rstd")
                nc.scalar.activation(rstd[:], lv[:], AF.Exp, scale=-0.5)
                al = stp.tile([128, CB, N], dt.float32, tag=prefix + "al", name=prefix + "al")
                be = stp.tile([128, CB, N], dt.float32, tag=prefix + "be", name=prefix + "be")
                tmp = stp.tile([128, CB, N], dt.float32, tag=prefix + "tmp", name=prefix + "tmp")
                nc.vector.tensor_mul(al[:], rstd[:], gc[:].to_broadcast((128, CB, N)))
                nc.vector.tensor_mul(tmp[:], mu, al[:])
                nc.vector.tensor_sub(be[:], bc[:].to_broadcast((128, CB, N)), tmp[:])
                return al, be

            def allreduce(stats, ar_tag):
                sin = dp.tile([128, 2 * CB * N], dt.float32, tag=ar_tag + "i", name=ar_tag + "i")
                sout = dp.tile([128, 2 * CB * N], dt.float32, tag=ar_tag + "o", name=ar_tag + "o")
                nc.gpsimd.dma_start(sin[:], stats[:])
                if for_sim:
                    nc.gpsimd.dma_start(sout[:], sin[:])
                else:
                    nc.gpsimd.collective_compute(
                        "AllReduce", AL.add,
                        replica_groups=[[0, 1, 2, 3], [4, 5, 6, 7]],
                        ins=[sin.opt()], outs=[sout.opt()],
                    )
                statsr = stp.tile([128, 2, CB, N], dt.float32, tag=ar_tag + "r", name=ar_tag + "r")
                nc.gpsimd.dma_start(statsr[:], sout[:])
                return statsr

            # ================= stage A: load x once, norm1 =================
            # xn as 12 half-tiles (cb, pixel half), group-major so a group's
            # 128 tokens are contiguous (matmul stationary needs 1 free dim);
            # groups 0-15 can start before the second half is normalized.
            xn_sb = [[xnp.tile([128, 16, N, 8], dt.bfloat16, tag=f"xn{cb}h{h}", name=f"xn{cb}h{h}")
                      for h in range(2)] for cb in range(CB)]

            with tc.tile_pool(name="xraw", bufs=1) as xp:
                x_sb = []
                stats1 = stp.tile([128, 2, CB, N], dt.float32, tag="n1st", name="n1st")
                for cb in range(CB):
                    xt = xp.tile([128, N, PX], dt.float32, tag=f"x{cb}", name=f"x{cb}")
                    srcv = xs[:, cb * 128:(cb + 1) * 128, :].rearrange("n c p -> c n p")
                    for q_ in range(2):
                        eng = nc.sync if q_ == 0 else nc.scalar
                        eng.dma_start(xt[:, q_ * 8:(q_ + 1) * 8], srcv[:, q_ * 8:(q_ + 1) * 8])
                    x_sb.append(xt)
                    # partial stats for this channel block
                    bn = stp.tile([128, N, 6], dt.float32, tag="n1bn", name="n1bn", bufs=2)
                    for n_ in range(N):
                        nc.vector.bn_stats(bn[:, n_], xt[:, n_])
                    bnv = bn[:].rearrange("c n (h s) -> c n h s", h=2)
                    t1 = stp.tile([128, N], dt.float32, tag="n1t1", name="n1t1", bufs=2)
                    nc.vector.tensor_add(t1[:], bnv[:, :, 0, 1], bnv[:, :, 1, 1])
                    nc.vector.tensor_scalar(stats1[:, 0, cb], t1[:], float(PX // 2), None, AL.mult)
                    m2 = stp.tile([128, N, 2], dt.float32, tag="n1m2", name="n1m2", bufs=2)
                    nc.scalar.activation(m2[:], bnv[:, :, :, 1], AF.Square)
                    t2 = stp.tile([128, N], dt.float32, tag="n1t2", name="n1t2", bufs=2)
                    nc.vector.tensor_add(t2[:], m2[:, :, 0], m2[:, :, 1])
                    t3 = stp.tile([128, N], dt.float32, tag="n1t3", name="n1t3", bufs=2)
                    nc.vector.tensor_add(t3[:], bnv[:, :, 0, 2], bnv[:, :, 1, 2])
                    nc.vector.tensor_scalar(t2[:], t2[:], float(PX // 2), None, AL.mult)
                    nc.vector.tensor_add(stats1[:, 1, cb], t3[:], t2[:])

                # weights ride the DMA queues behind x
                wq_sb = []
                for kc in range(CB):
                    t = wp.tile([128, CO], dt.bfloat16, tag=f"wq{kc}", name=f"wq{kc}")
                    (nc.sync if kc % 2 == 0 else nc.scalar).dma_start(t[:], wq[kc * 128:(kc + 1) * 128, :])
                    wq_sb.append(t)

                statsr1 = allreduce(stats1, "ar1")
                al1, be1 = norm_coeffs(statsr1, g1c, b1c, 1.0 / (4 * PX), "n1")

                # normalize, pixel-half major so groups can start early
                engs = [nc.vector, nc.vector, nc.scalar, nc.gpsimd]
                ei = 0
                for h in range(2):
                    for cb in range(CB):
                        for n in range(N):
                            a_ap = al1[:, cb, n:n + 1]
                            b_ap = be1[:, cb, n:n + 1]
                            src = x_sb[cb][:, n, h * 128:(h + 1) * 128].rearrange(
                                "c (g p) -> c g p", g=16)
                            dst = xn_sb[cb][h][:, :, n, :]
                            e = engs[ei % 4]
                            ei += 1
                            if e is nc.scalar:
                                nc.scalar.activation(dst, src, AF.Identity, bias=b_ap, scale=a_ap)
                            else:
                                e.tensor_scalar(dst, src, a_ap, b_ap, AL.mult, AL.add)

            # w2 loads ride the idle DMA track once the head burst is done
            w2_sb = []
            for kc in range(CB):
                t = wp.tile([128, EMB], dt.bfloat16, tag=f"w2{kc}", name=f"w2{kc}")
                nc.gpsimd.dma_start(t[:], w2[kc * 128:(kc + 1) * 128, :])
                w2_sb.append(t)

            # ============ stage B: qkv + attention (32 groups) ============
            yp_cm = tc.tile_pool(name="ybuf", bufs=1)
            yp = yp_cm.__enter__()
            y1 = yp.tile([128, CB, N, PX], dt.bfloat16, tag="y1", name="y1")

            with tc.tile_pool(name="ystat", bufs=1, space="PSUM") as ysp, \
                 tc.tile_pool(name="qps", bufs=2, space="PSUM") as qpp, \
                 tc.tile_pool(name="qkTps", bufs=1, space="PSUM") as qkTp, \
                 tc.tile_pool(name="sTps", bufs=1, space="PSUM") as sTp, \
                 tc.tile_pool(name="aoTps", bufs=1, space="PSUM") as aoTp, \
                 tc.tile_pool(name="attw", bufs=3) as ap_, \
                 tc.tile_pool(name="attw3", bufs=3) as ap3:

                # one persistent PSUM tile: norm2-stat accumulators in cols
                # 0:192, the per-j AV output scratch in cols 192:452
                opool = ysp.tile([128, 452], dt.float32, tag="opool", name="opool")
                ystat_ps = opool[:, 0:192].rearrange("c (s cb n) -> c s cb n", s=2, cb=CB)

                def qkv_phase(g):
                    """qkv matmuls + evictions + LN stats/combine/apply for group g.
                    Returns (q_sb, k_sb, v_sb) with q_hat/k_hat applied in place."""
                    gh, gl = divmod(g, 16)
                    # q/k stored per pair-block with the two heads interleaved
                    # along columns: bn_stats segments an AP into its even and
                    # odd elements, so per-head stats come straight out.
                    q_sb = ap_.tile([128, CB, 64, 2], dt.bfloat16, tag="q", name="q")
                    k_sb = ap_.tile([128, CB, 64, 2], dt.bfloat16, tag="k", name="k")
                    v_sb = ap_.tile([128, HEADS, 65], dt.bfloat16, tag="v", name="v")
                    nc.vector.memset(v_sb[:, :, 64:65], 1.0)
                    bnq = stp.tile([128, 6, 6], dt.float32, tag="bnq", name="bnq")
                    bnk = stp.tile([128, 6, 6], dt.float32, tag="bnk", name="bnk")
                    for hp in range(6):
                        qp = qpp.tile([128, 384], dt.float32, tag="qp", name="qp")
                        for kc in range(CB):
                            nc.tensor.matmul(qp[:], xn_sb[kc][gh][:, gl],
                                             wq_sb[kc][:, hp * 384:(hp + 1) * 384],
                                             start=(kc == 0), stop=(kc == CB - 1))
                        qpv = qp[:].rearrange("c (h qkv e) -> c h qkv e", h=2, qkv=3)
                        # batched strided evictions (gpsimd has no PSUM port);
                        # q/k interleave the pair: head t lands at columns t::2
                        for qkv_i in range(3):
                            if qkv_i == 2:
                                dap = v_sb[:, 2 * hp:2 * hp + 2, 0:64]
                            else:
                                d = (q_sb, k_sb)[qkv_i]
                                dap = d[:, hp].rearrange("c e t -> c t e")
                            if (hp + qkv_i) % 2 == 0:
                                nc.vector.tensor_copy(dap, qpv[:, :, qkv_i])
                            else:
                                nc.scalar.copy(dap, qpv[:, :, qkv_i])
                    # per-head LN stats: bn_stats over [128, 3, 128] views segments
                    # each 128-col pair into its two heads: per head (cnt, mean, M2)
                    for p in range(CB):
                        nc.vector.bn_stats(bnq[:, p], q_sb[:, p].rearrange("c e t -> c (e t)"))
                        nc.vector.bn_stats(bnk[:, p], k_sb[:, p].rearrange("c e t -> c (e t)"))
                    bq = bnq[:].rearrange("c p (s f) -> c (p s) f", s=2)
                    bk = bnk[:].rearrange("c p (s f) -> c (p s) f", s=2)
                    # rstd = exp(-0.5*ln(M2/HD + eps))
                    lvq = stp.tile([128, 12], dt.float32, tag="lvq", name="lvq")
                    nc.scalar.activation(lvq[:], bq[:, :, 2], AF.Ln, bias=eps_c[:], scale=1.0 / HD)
                    rsq = stp.tile([128, 12], dt.float32, tag="rsq", name="rsq")
                    nc.scalar.activation(rsq[:], lvq[:], AF.Exp, scale=-0.5)
                    lvk = stp.tile([128, 12], dt.float32, tag="lvk", name="lvk")
                    nc.scalar.activation(lvk[:], bk[:, :, 2], AF.Ln, bias=eps_c[:], scale=1.0 / HD)
                    rsk = stp.tile([128, 12], dt.float32, tag="rsk", name="rsk")
                    nc.scalar.activation(rsk[:], lvk[:], AF.Exp, scale=-0.5)
                    mrq = stp.tile([128, 12], dt.float32, tag="mrq", name="mrq")
                    nc.vector.scalar_tensor_tensor(mrq[:], bq[:, :, 1], -1.0, rsq[:],
                                                   AL.mult, AL.mult)
                    # apply: q_hat = (q - mu_q)*rstd_q ; k_hat = k*rstd_k
                    # (k centering is unnecessary once q is centered)
                    for h in range(HEADS):
                        qap = q_sb[:, h // 2, :, h % 2]
                        kap = k_sb[:, h // 2, :, h % 2]
                        nc.gpsimd.tensor_scalar(qap, qap,
                                                rsq[:, h:h + 1], mrq[:, h:h + 1], AL.mult, AL.add)
                        nc.gpsimd.tensor_scalar(kap, kap,
                                                rsk[:, h:h + 1], None, AL.mult)
                    return q_sb, k_sb, v_sb

                def attn_phase(g, q_sb, k_sb, v_sb):
                    """transposes, scores, softmax, AV, normalize for group g.

                    Per-head transposes into [64, .] tiles keep every matmul
                    operand at base partition 0 (nonzero PE tile_position dies
                    at runtime). The AV output reuses the consumed scores tile
                    (same PSUM bank), and j-blocks cycle two buffers so exp
                    overlaps the next block's scores."""
                    qkTs = []
                    for j in range(3):
                        qkT = qkTp.tile([64, 1024], dt.bfloat16, tag="qkT", name="qkT", bufs=2)
                        for hh in range(4):
                            h = 4 * j + hh
                            o = hh * 256
                            nc.tensor.transpose(qkT[:, o:o + 128], q_sb[:, h // 2, :, h % 2], id_sb[:])
                            nc.tensor.transpose(qkT[:, o + 128:o + 256], k_sb[:, h // 2, :, h % 2], id_sb[:])
                        qs = ap3.tile([64, 1024], dt.bfloat16, tag=f"qkTs{j}", name=f"qkTs{j}", bufs=2)
                        e0, e1 = ((nc.vector, nc.scalar), (nc.scalar, nc.vector))[j % 2]
                        if e0 is nc.vector:
                            nc.vector.tensor_copy(qs[:, 0:512], qkT[:, 0:512])
                            nc.scalar.copy(qs[:, 512:1024], qkT[:, 512:1024])
                        else:
                            nc.scalar.copy(qs[:, 0:512], qkT[:, 0:512])
                            nc.vector.tensor_copy(qs[:, 512:1024], qkT[:, 512:1024])
                        qkTs.append(qs)

                    ao = ap3.tile([128, HEADS, 64], dt.bfloat16, tag="ao", name="ao")

                    def scores(j):
                        qs = qkTs[j]
                        sT = sTp.tile([128, 512], dt.float32, tag="sT", name="sT", bufs=2)
                        for hh in range(4):
                            o = hh * 256
                            nc.tensor.matmul(sT[:, hh * 128:(hh + 1) * 128],
                                             qs[:, o + 128:o + 256], qs[:, o:o + 128],
                                             start=True, stop=False)
                            # rank-9 additive mask: -C on cross-pixel pairs, so
                            # exp() masks them without a separate multiply
                            nc.tensor.matmul(sT[:, hh * 128:(hh + 1) * 128],
                                             mk9_sb[:, 0:128], mk9_sb[:, 128:256],
                                             start=False, stop=True)
                        u4 = ap3.tile([128, 512], dt.bfloat16, tag=f"u4{j}", name=f"u4{j}", bufs=2)
                        nc.scalar.activation(u4[:], sT[:], AF.Exp, scale=SCALE)
                        return sT, u4

                    def av(j, sT, u4):
                        # reuse the consumed scores tile for the AV output
                        o24 = sT[:, 0:260].rearrange("c (hh e) -> c hh e", hh=4)
                        for hh in range(4):
                            h = 4 * j + hh
                            nc.tensor.matmul(o24[:, hh], u4[:, hh * 128:(hh + 1) * 128],
                                             v_sb[:, h, :], start=True, stop=True)
                        rd = stp.tile([128, 4], dt.float32, tag="rd", name="rd", bufs=3)
                        nc.vector.reciprocal(rd[:], o24[:, :, 64])
                        nc.vector.tensor_tensor(ao[:, 4 * j:4 * j + 4, :], o24[:, :, 0:64],
                                                rd[:].to_broadcast((128, 4, 64)), AL.mult)

                    s0 = scores(0)
                    s1 = scores(1)
                    av(0, *s0)
                    s2 = scores(2)
                    av(1, *s1)
                    av(2, *s2)
                    ao2 = ap3.tile([128, HEADS, 64], dt.bfloat16, tag="ao2", name="ao2")
                    nc.vector.tensor_mul(ao2[:], ao[:], ao[:])
                    return ao, ao2

                def tail_phase(g, ao, ao2):
                    """norm2 stat accumulation + transpose back to channel-major y."""
                    gsl = slice(g * 8, (g + 1) * 8)
                    # start=True arms the whole 2KB psum bank as pending-zero;
                    # only the very first matmul may set it, later regions'
                    # first writes then store instead of accumulating.
                    last = (g == NG - 1)
                    for p in range(CB):
                        nc.tensor.matmul(ystat_ps[:, 0, p], ao[:, 2 * p:2 * p + 2, :], sel_sb[:],
                                         start=(g == 0 and p == 0), stop=last,
                                         skip_group_check=True)
                        nc.tensor.matmul(ystat_ps[:, 1, p], ao2[:, 2 * p:2 * p + 2, :], sel_sb[:],
                                         start=False, stop=last, skip_group_check=True)
                    aoT = aoTp.tile([128, 768], dt.bfloat16, tag="aoT", name="aoT")
                    for p in range(CB):
                        nc.tensor.transpose(aoT[:, p * 128:(p + 1) * 128],
                                            ao[:, 2 * p:2 * p + 2, :], id_sb[:])
                    for jt in range(3):
                        src = aoT[:, jt * 256:(jt + 1) * 256].rearrange(
                            "c (cb n p) -> c cb n p", cb=2, n=N)
                        dst = y1[:, 2 * jt:2 * jt + 2, :, gsl]
                        if (g + jt) % 2 == 0:
                            nc.vector.tensor_copy(dst, src)
                        else:
                            nc.scalar.copy(dst, src)

                # 2-stage software pipeline: group g's LN chain (DVE/ACT/Pool)
                # completes while the PE runs attention for group g-1.
                prev = None
                for g in range(NG):
                    cur = qkv_phase(g)
                    if prev is not None:
                        pg, pq, pk, pv = prev
                        ao, ao2 = attn_phase(pg, pq, pk, pv)
                        tail_phase(pg, ao, ao2)
                    prev = (g,) + cur
                pg, pq, pk, pv = prev
                ao, ao2 = attn_phase(pg, pq, pk, pv)
                tail_phase(pg, ao, ao2)

                # gather norm2 stats from the accumulator
                stats2 = stp.tile([128, 2, CB, N], dt.float32, tag="n2st", name="n2st")
                nc.vector.tensor_copy(stats2[:], ystat_ps)

            statsr2 = allreduce(stats2, "ar2")
            al2, be2 = norm_coeffs(statsr2, g2c, b2c, 1.0 / (4 * PX), "n2")

            # ============ stage C: norm2 apply + output projection ============
            with tc.tile_pool(name="ynb", bufs=2) as ynp, \
                 tc.tile_pool(name="opps", bufs=4, space="PSUM") as opp, \
                 tc.tile_pool(name="obuf", bufs=4) as op_:
                for ch in range(8):           # 2-token chunks
                    yn_sb = [ynp.tile([128, 2, PX], dt.bfloat16, tag=f"yn{t}", name=f"yn{t}")
                             for t in range(CB)]
                    for cb in range(CB):
                        for ni in range(2):
                            nn = 2 * ch + ni
                            a_ap = al2[:, cb, nn:nn + 1]
                            b_ap = be2[:, cb, nn:nn + 1]
                            e = (nc.vector, nc.scalar, nc.vector, nc.gpsimd)[(cb + nn) % 4]
                            if e is nc.scalar:
                                nc.scalar.activation(yn_sb[cb][:, ni], y1[:, cb, nn],
                                                     AF.Identity, bias=b_ap, scale=a_ap)
                            else:
                                e.tensor_scalar(yn_sb[cb][:, ni], y1[:, cb, nn],
                                                a_ap, b_ap, AL.mult, AL.add)
                    for mt in range(CB):
                        op = opp.tile([128, 512], dt.float32, tag="op", name="op")
                        for kc in range(CB):
                            nc.tensor.matmul(op[:], w2_sb[kc][:, mt * 128:(mt + 1) * 128],
                                             yn_sb[kc][:],
                                             start=(kc == 0), stop=(kc == CB - 1))
                        osb = op_.tile([128, 2, PX], dt.float32, tag="osb", name="osb")
                        srcv = op[:].rearrange("c (n p) -> c n p", n=2)
                        if mt % 2 == 0:
                            nc.vector.tensor_copy(osb[:], srcv)
                        else:
                            nc.scalar.copy(osb[:], srcv)
                        dma_e = (nc.sync, nc.scalar)[(ch + mt) % 2]
                        dma_e.dma_start(out[2 * ch:2 * ch + 2, mt * 128:(mt + 1) * 128, :]
                                        .rearrange("n c p -> c n p"), osb[:])
            yp_cm.__exit__(None, None, None)
            xnp_cm.__exit__(None, None, None)

    nc.finalize()
    return nc


def _build_program(fast, ln_affine, asf, for_sim=False):
    if fast:
        return _build_fast(for_sim=for_sim)
    return _build_general(ln_affine, asf, for_sim=for_sim)


def _build_general(ln_affine, asf, for_sim=False):
    """asf: None for the fast path (attn_scale_factor == 1), else tuple of 12 floats."""
    import concourse.bacc as bacc
    import concourse.mybir as mybir
    import concourse.tile as tile

    _pin_act_tables()

    dt = mybir.dt
    AF = mybir.ActivationFunctionType
    AL = mybir.AluOpType
    AX = mybir.AxisListType.X

    nc = bacc.Bacc("TRN2", target_bir_lowering=False, debug=False, num_devices=NCORES)

    def din(name, shape, d=dt.float32):
        return nc.dram_tensor(name, list(shape), d, kind="ExternalInput").ap()

    xs = din("xs", (N, EMB, PX))
    wq = din("wq", (EMB, CO), dt.bfloat16)        # W_qkv^T
    bq = din("bq", (1, CO), dt.bfloat16)
    w2 = din("w2", (EMB, EMB), dt.bfloat16)       # W_out^T
    b2r = din("b2r", (1, EMB), dt.bfloat16)       # b_out
    n1w = din("n1w", (EMB,))
    n1b = din("n1b", (EMB,))
    n2w = din("n2w", (EMB,))
    n2b = din("n2b", (EMB,))
    ident = din("ident", (128, 128), dt.bfloat16)
    mask4 = din("mask4", (128, 512), dt.bfloat16)
    if ln_affine:
        qgw = din("qgw", (128, HD), dt.bfloat16)  # qnorm_w replicated over partitions
        qgb = din("qgb", (128, HD), dt.bfloat16)
        kgw = din("kgw", (128, HD), dt.bfloat16)
        kgb = din("kgb", (128, HD), dt.bfloat16)
    if asf is not None:
        bsel = din("bsel", (128, 8), dt.bfloat16)    # sel[t,p] = (t%8==p)
        bselT = din("bselT", (8, 128), dt.bfloat16)
    out = nc.dram_tensor("out", [N, EMB, PX], dt.float32, kind="ExternalOutput").ap()

    with tile.TileContext(nc) as tc:
        with tc.tile_pool(name="const", bufs=1) as cp, \
             tc.tile_pool(name="wts", bufs=1) as wp, \
             tc.tile_pool(name="xnyn", bufs=6) as xnp, \
             tc.tile_pool(name="dram", bufs=1, space="DRAM") as dp, \
             tc.tile_pool(name="stats", bufs=2) as stp:

            # ---- constants ----
            id_sb = cp.tile([128, 128], dt.bfloat16)
            nc.sync.dma_start(id_sb[:], ident[:])
            mk_sb = cp.tile([128, 512], dt.bfloat16)
            nc.sync.dma_start(mk_sb[:], mask4[:])
            ones_r = cp.tile([1, 512], dt.bfloat16)
            nc.vector.memset(ones_r[:], 1.0)
            ones_c = cp.tile([128, 1], dt.bfloat16)
            nc.vector.memset(ones_c[:], 1.0)
            eps_c = cp.tile([128, 1], dt.float32)
            nc.vector.memset(eps_c[:], EPS)
            g1c = cp.tile([128, CB], dt.float32)
            nc.sync.dma_start(g1c[:], n1w.rearrange("(cb c) -> c cb", c=128))
            b1c = cp.tile([128, CB], dt.float32)
            nc.sync.dma_start(b1c[:], n1b.rearrange("(cb c) -> c cb", c=128))
            g2c = cp.tile([128, CB], dt.float32)
            nc.sync.dma_start(g2c[:], n2w.rearrange("(cb c) -> c cb", c=128))
            b2c = cp.tile([128, CB], dt.float32)
            nc.sync.dma_start(b2c[:], n2b.rearrange("(cb c) -> c cb", c=128))
            bq_sb = cp.tile([1, CO], dt.bfloat16)
            nc.sync.dma_start(bq_sb[:], bq[:])
            b2_sb = cp.tile([1, EMB], dt.bfloat16)
            nc.sync.dma_start(b2_sb[:], b2r[:])
            if ln_affine:
                qgw_sb = cp.tile([128, HD], dt.bfloat16)
                nc.sync.dma_start(qgw_sb[:], qgw[:])
                qgb_sb = cp.tile([128, HD], dt.bfloat16)
                nc.sync.dma_start(qgb_sb[:], qgb[:])
                kgw_sb = cp.tile([128, HD], dt.bfloat16)
                nc.sync.dma_start(kgw_sb[:], kgw[:])
                kgb_sb = cp.tile([128, HD], dt.bfloat16)
                nc.sync.dma_start(kgb_sb[:], kgb[:])
            if asf is not None:
                bsel_sb = cp.tile([128, 8], dt.bfloat16)
                nc.sync.dma_start(bsel_sb[:], bsel[:])
                bselT_sb = cp.tile([8, 128], dt.bfloat16)
                nc.sync.dma_start(bselT_sb[:], bselT[:])

            wq_sb = []
            for kc in range(CB):
                t = wp.tile([128, CO], dt.bfloat16, tag=f"wq{kc}", name=f"wq{kc}")
                nc.sync.dma_start(t[:], wq[kc * 128:(kc + 1) * 128, :])
                wq_sb.append(t)
            w2_sb = []
            for kc in range(CB):
                t = wp.tile([128, EMB], dt.bfloat16, tag=f"wq{kc}", name=f"w2{kc}")
                nc.sync.dma_start(t[:], w2[kc * 128:(kc + 1) * 128, :])
                w2_sb.append(t)


            def norm_coeffs(statsr, gc, bc, inv_count, prefix):
                """statsr (128, 2, CB, N) summed stats -> alpha,beta (128, CB, N)."""
                mue2 = stp.tile([128, 2, CB, N], dt.float32, tag=prefix + "mu", name=prefix + "mu")
                nc.vector.tensor_scalar(mue2[:], statsr[:], inv_count, None, AL.mult)
                mu = mue2[:, 0]
                e2 = mue2[:, 1]
                msq = stp.tile([128, CB, N], dt.float32, tag=prefix + "msq", name=prefix + "msq")
                nc.scalar.activation(msq[:], mu, AF.Square)
                var = stp.tile([128, CB, N], dt.float32, tag=prefix + "var", name=prefix + "var")
                nc.vector.tensor_sub(var[:], e2, msq[:])
                # rstd = exp(-0.5*ln(var+eps)) -- keeps ACT in the exp/ln table set
                lv = stp.tile([128, CB, N], dt.float32, tag=prefix + "lv", name=prefix + "lv")
                nc.scalar.activation(lv[:], var[:], AF.Ln, bias=eps_c[:])
                rstd = stp.tile([128, CB, N], dt.float32, tag=prefix + "rstd", name=prefix + "rstd")
                nc.scalar.activation(rstd[:], lv[:], AF.Exp, scale=-0.5)
                al = stp.tile([128, CB, N], dt.float32, tag=prefix + "al", name=prefix + "al")
                be = stp.tile([128, CB, N], dt.float32, tag=prefix + "be", name=prefix + "be")
                tmp = stp.tile([128, CB, N], dt.float32, tag=prefix + "tmp", name=prefix + "tmp")
                nc.vector.tensor_mul(al[:], rstd[:], gc[:].to_broadcast((128, CB, N)))
                nc.vector.tensor_mul(tmp[:], mu, al[:])
                nc.vector.tensor_sub(be[:], bc[:].to_broadcast((128, CB, N)), tmp[:])
                return al, be

            def inorm_stats(src_tiles, prefix, ar_tag, lazy=False):
                """instance-norm partial stats + AllReduce -> (sum, sumsq).

                Sums via DVE reduce; sum-of-squares via ACT Square with
                accum_out (keeps the idle engine busy in this phase)."""
                stats = stp.tile([128, 2, CB, N], dt.float32, tag=prefix + "st", name=prefix + "st")
                for cb in range(CB):
                    st = src_tiles[cb]
                    bn = stp.tile([128, N, 6], dt.float32, tag=prefix + "bn", name=prefix + "bn", bufs=2)
                    for n_ in range(N):
                        nc.vector.bn_stats(bn[:, n_], st[:, n_])
                    bnv = bn[:].rearrange("c n (h s) -> c n h s", h=2)
                    t1 = stp.tile([128, N], dt.float32, tag=prefix + "t1", name=prefix + "t1", bufs=2)
                    nc.vector.tensor_add(t1[:], bnv[:, :, 0, 1], bnv[:, :, 1, 1])
                    nc.vector.tensor_scalar(stats[:, 0, cb], t1[:], float(PX // 2), None, AL.mult)
                    m2 = stp.tile([128, N, 2], dt.float32, tag=prefix + "m2", name=prefix + "m2", bufs=2)
                    nc.scalar.activation(m2[:], bnv[:, :, :, 1], AF.Square)
                    t2 = stp.tile([128, N], dt.float32, tag=prefix + "t2", name=prefix + "t2", bufs=2)
                    nc.vector.tensor_add(t2[:], m2[:, :, 0], m2[:, :, 1])
                    t3 = stp.tile([128, N], dt.float32, tag=prefix + "t3", name=prefix + "t3", bufs=2)
                    nc.vector.tensor_add(t3[:], bnv[:, :, 0, 2], bnv[:, :, 1, 2])
                    nc.vector.tensor_scalar(t2[:], t2[:], float(PX // 2), None, AL.mult)
                    nc.vector.tensor_add(stats[:, 1, cb], t3[:], t2[:])
                sin = dp.tile([128, 2 * CB * N], dt.float32, tag=ar_tag + "i", name=ar_tag + "i")
                sout = dp.tile([128, 2 * CB * N], dt.float32, tag=ar_tag + "o", name=ar_tag + "o")
                nc.gpsimd.dma_start(sin[:], stats[:])
                if for_sim:
                    nc.gpsimd.dma_start(sout[:], sin[:])
                else:
                    nc.gpsimd.collective_compute(
                        "AllReduce", AL.add,
                        replica_groups=[[0, 1, 2, 3], [4, 5, 6, 7]],
                        ins=[sin.opt()], outs=[sout.opt()],
                    )
                statsr = stp.tile([128, 2, CB, N], dt.float32, tag=prefix + "str", name=prefix + "str")
                nc.gpsimd.dma_start(statsr[:], sout[:])
                return statsr

            # ================= stage A: load x, norm1 =================
            xn_sb = []
            with tc.tile_pool(name="xraw", bufs=2) as xp:
                def load_x(cb):
                    xt = xp.tile([128, N, PX], dt.float32, tag="x", name="x")
                    srcv = xs[:, cb * 128:(cb + 1) * 128, :].rearrange("n c p -> c n p")
                    for q_ in range(4):
                        eng = nc.sync if q_ % 2 == 0 else nc.scalar
                        eng.dma_start(xt[:, q_ * 4:(q_ + 1) * 4], srcv[:, q_ * 4:(q_ + 1) * 4])
                    return xt
                statsr = inorm_stats([load_x(cb) for cb in range(CB)], "n1", "ar1", lazy=True)
                al1, be1 = norm_coeffs(statsr, g1c, b1c, 1.0 / (4 * PX), "n1")
                for cb in range(CB):
                    xt = load_x(cb)
                    xn = xnp.tile([128, NG, N, 8], dt.bfloat16, tag="xnyn", name="xnyn")
                    for n in range(N):
                        a_ap = al1[:, cb, n:n + 1]
                        b_ap = be1[:, cb, n:n + 1]
                        src_ap = xt[:, n].rearrange("c (g p) -> c g p", g=NG)
                        if n % 3 != 2:
                            nc.vector.tensor_scalar(xn[:, :, n], src_ap, a_ap, b_ap, AL.mult, AL.add)
                        else:
                            nc.scalar.activation(xn[:, :, n], src_ap, AF.Identity, bias=b_ap, scale=a_ap)
                    xn_sb.append(xn)

            # ============ stages B-D: qkv + attention ============
            yp_cm = tc.tile_pool(name="ybuf", bufs=1)
            yp = yp_cm.__enter__()
            y1 = yp.tile([128, CB, N, PX], dt.bfloat16, tag="y1", name="y1")
            with tc.tile_pool(name="qkvps", bufs=2, space="PSUM") as qkvp, \
                 tc.tile_pool(name="qkTps", bufs=1, space="PSUM") as qkTp, \
                 tc.tile_pool(name="sT4ps", bufs=2, space="PSUM") as sT4p, \
                 tc.tile_pool(name="o24ps", bufs=1, space="PSUM") as o24p, \
                 tc.tile_pool(name="aoTps", bufs=2, space="PSUM") as aoTp, \
                 tc.tile_pool(name="attw", bufs=3) as ap_, \
                 tc.tile_pool(name="attw3", bufs=4) as ap3:

                for g in range(NG):
                    gsl = slice(g * 8, (g + 1) * 8)
                    qkvg = ap_.tile([128, HEADS, 196], dt.bfloat16, tag="qkvg", name="qkvg")
                    nc.vector.memset(qkvg[:, :, 192:193], 1.0)
                    bnq = stp.tile([128, HEADS, 6], dt.float32, tag="bnq", name="bnq")
                    bnk = stp.tile([128, HEADS, 6], dt.float32, tag="bnk", name="bnk")
                    for hp in range(6):
                        qp = qkvp.tile([128, 384], dt.float32, tag="qkvps", name="qkvps")
                        for kc in range(CB):
                            nc.tensor.matmul(qp[:], xn_sb[kc][:, g], wq_sb[kc][:, hp * 384:(hp + 1) * 384],
                                             start=(kc == 0), stop=False)
                        nc.tensor.matmul(qp[:], ones_r[0:1, 0:128], bq_sb[0:1, hp * 384:(hp + 1) * 384],
                                         start=False, stop=True)
                        qpv = qp[:].rearrange("c (h e) -> c h e", h=2)
                        nc.scalar.copy(qkvg[:, 2 * hp:2 * hp + 2, 0:192], qpv)
                        for hh_ in (2 * hp, 2 * hp + 1):
                            nc.vector.bn_stats(bnq[:, hh_], qkvg[:, hh_, 0:64])
                            nc.vector.bn_stats(bnk[:, hh_], qkvg[:, hh_, 64:128])

                    # combine bn_stats -> rstd, -mu*rstd  (batched q,k per group)
                    rs = {}
                    nm = {}
                    for qk, bn in (("q", bnq), ("k", bnk)):
                        bnv = bn[:].rearrange("c h (e s) -> c h e s", e=2)
                        d = stp.tile([128, HEADS], dt.float32, tag="lnd" + qk, name="lnd" + qk)
                        nc.vector.tensor_sub(d[:], bnv[:, :, 0, 1], bnv[:, :, 1, 1])
                        d2 = stp.tile([128, HEADS], dt.float32, tag="lnd2" + qk, name="lnd2" + qk)
                        nc.scalar.activation(d2[:], d[:], AF.Square)
                        m2 = stp.tile([128, HEADS], dt.float32, tag="lnm2" + qk, name="lnm2" + qk)
                        nc.vector.tensor_add(m2[:], bnv[:, :, 0, 2], bnv[:, :, 1, 2])
                        nc.vector.tensor_scalar(d2[:], d2[:], float(HD) / 4.0, None, AL.mult)
                        nc.vector.tensor_add(m2[:], m2[:], d2[:])
                        # rstd = exp(-0.5*ln(m2/HD + eps))
                        lv = stp.tile([128, HEADS], dt.float32, tag="lnlv" + qk, name="lnlv" + qk)
                        nc.scalar.activation(lv[:], m2[:], AF.Ln, bias=eps_c[:], scale=1.0 / HD)
                        rst = stp.tile([128, HEADS], dt.float32, tag="lnrs" + qk, name="lnrs" + qk)
                        nc.scalar.activation(rst[:], lv[:], AF.Exp, scale=-0.5)
                        nmu = stp.tile([128, HEADS], dt.float32, tag="lnnm" + qk, name="lnnm" + qk)
                        nc.vector.tensor_add(nmu[:], bnv[:, :, 0, 1], bnv[:, :, 1, 1])
                        nc.vector.tensor_scalar(nmu[:], nmu[:], -0.5, None, AL.mult)
                        nc.vector.tensor_mul(nmu[:], nmu[:], rst[:])
                        rs[qk] = rst
                        nm[qk] = nmu

                    for h in range(HEADS):
                        j = h % 4
                        qsl = qkvg[:, h, 0:64]
                        ksl = qkvg[:, h, 64:128]
                        qkn = ap3.tile([128, 128], dt.bfloat16, tag="qkn", name="qkn")
                        nc.gpsimd.tensor_scalar(qkn[:, 0:64], qsl, rs["q"][:, h:h + 1],
                                                nm["q"][:, h:h + 1], AL.mult, AL.add)
                        nc.gpsimd.tensor_scalar(qkn[:, 64:128], ksl, rs["k"][:, h:h + 1],
                                                nm["k"][:, h:h + 1], AL.mult, AL.add)
                        if ln_affine:
                            nc.vector.tensor_mul(qkn[:, 0:64], qkn[:, 0:64], qgw_sb[:])
                            nc.vector.tensor_add(qkn[:, 0:64], qkn[:, 0:64], qgb_sb[:])
                            nc.vector.tensor_mul(qkn[:, 64:128], qkn[:, 64:128], kgw_sb[:])
                            nc.vector.tensor_add(qkn[:, 64:128], qkn[:, 64:128], kgb_sb[:])
                        if h % 2 == 0:
                            qkT = qkTp.tile([64, 512], dt.bfloat16, tag="qkT", name="qkT")
                        off = (h % 2) * 256
                        nc.tensor.transpose(qkT[:, off:off + 128], qkn[:, 0:64], id_sb[:])
                        nc.tensor.transpose(qkT[:, off + 128:off + 256], qkn[:, 64:128], id_sb[:])
                        if h % 2 == 1:
                            qkTs = ap3.tile([64, 512], dt.bfloat16, tag="qkTs", name="qkTs")
                            if h % 4 == 1:
                                nc.vector.tensor_copy(qkTs[:], qkT[:])
                            else:
                                nc.scalar.copy(qkTs[:], qkT[:])
                            if h % 4 == 1:
                                sT4 = sT4p.tile([128, 512], dt.float32, tag="sT4", name="sT4")
                            for hv in (h - 1, h):
                                jv = hv % 4
                                o = (hv % 2) * 256
                                nc.tensor.matmul(sT4[:, jv * 128:(jv + 1) * 128],
                                                 qkTs[:, o + 128:o + 256], qkTs[:, o:o + 128],
                                                 start=True, stop=True)
                        if j == 3:
                            u4 = ap_.tile([128, 512], dt.bfloat16, tag="u4", name="u4")
                            nc.scalar.activation(u4[:], sT4[:], AF.Exp, scale=SCALE)
                            um4 = ap_.tile([128, 512], dt.bfloat16, tag="um4", name="um4")
                            nc.vector.tensor_mul(um4[:], u4[:], mk_sb[:])
                            o24 = o24p.tile([128, 260], dt.float32, tag="o24", name="o24")
                            for jj in range(4):
                                hh = h - 3 + jj
                                usl = um4[:, jj * 128:(jj + 1) * 128]
                                nc.tensor.matmul(o24[:, jj * 65:jj * 65 + 65], usl, qkvg[:, hh, 128:193],
                                                 start=True, stop=True)
                            rd = stp.tile([128, 4], dt.float32, tag="rd", name="rd")
                            nc.vector.reciprocal(rd[:], o24[:].rearrange("c (j e) -> c j e", e=65)[:, :, 64])
                            aoT = aoTp.tile([128, 256], dt.bfloat16, tag="aoT", name="aoT")
                            for jj in range(4):
                                hh = h - 3 + jj
                                if asf is None:
                                    ao_t = ap3.tile([128, 64], dt.bfloat16, tag="ao", name="ao")
                                    ao = ao_t[:]
                                    nc.vector.tensor_scalar(ao, o24[:, jj * 65:jj * 65 + 64],
                                                            rd[:, jj:jj + 1], None, AL.mult)
                                else:
                                    ao = None
                                    ao_t = ap3.tile([128, 64], dt.bfloat16, tag="ao", name="ao")
                                    ao = ao_t[:]
                                    s_h = float(asf[hh])
                                    nc.vector.tensor_scalar(ao, o24[:, jj * 65:jj * 65 + 64],
                                                            rd[:, jj:jj + 1], s_h, AL.mult, AL.mult)
                                    vsp = o24p.tile([8, 65], dt.float32, tag="vsp", name="vsp")
                                    nc.tensor.matmul(vsp[:, 0:64], bsel_sb[:], qkvg[:, hh, 128:192],
                                                     start=True, stop=True)
                                    vss = ap3.tile([8, 64], dt.bfloat16, tag="vss", name="vss")
                                    nc.vector.tensor_copy(vss[:], vsp[:, 0:64])
                                    vrp = o24p.tile([128, 65], dt.float32, tag="vrp", name="vrp")
                                    nc.tensor.matmul(vrp[:, 0:64], bselT_sb[:], vss[:],
                                                     start=True, stop=True)
                                    vcor = ap3.tile([128, 64], dt.bfloat16, tag="vcor", name="vcor")
                                    nc.vector.tensor_scalar(vcor[:], vrp[:, 0:64],
                                                            (1.0 - s_h) / N, None, AL.mult)
                                    nc.vector.tensor_add(ao, ao, vcor[:])
                                half = hh % 2
                                col = jj // 2
                                nc.tensor.transpose(aoT[half * 64:half * 64 + 64, col * 128:(col + 1) * 128],
                                                    ao, id_sb[:])
                            for jj in range(4):
                                hh = h - 3 + jj
                                half, col = hh % 2, jj // 2
                                src = aoT[half * 64:half * 64 + 64,
                                          col * 128:(col + 1) * 128].rearrange("c (n p) -> c n p", n=N)
                                dst = y_sb[hh // 2][half * 64:half * 64 + 64, :, gsl]
                                if jj % 2 == 0:
                                    nc.vector.tensor_copy(dst, src)
                                else:
                                    nc.scalar.copy(dst, src)

            # ================= stage E: norm2 + out-proj =================
            statsr2 = inorm_stats(y_sb, "n2", "ar2")
            al2, be2 = norm_coeffs(statsr2, g2c, b2c, 1.0 / (4 * PX), "n2")
            yn_sb = []
            for cb in range(CB):
                yn = xnp.tile([128, N, PX], dt.bfloat16, tag="xnyn", name="xnyn")
                for n in range(N):
                    a_ap = al2[:, cb, n:n + 1]
                    b_ap = be2[:, cb, n:n + 1]
                    if n % 2 == 0:
                        nc.vector.tensor_scalar(yn[:, n], y_sb[cb][:, n], a_ap, b_ap, AL.mult, AL.add)
                    else:
                        nc.scalar.activation(yn[:, n], y_sb[cb][:, n], AF.Identity, bias=b_ap, scale=a_ap)
                yn_sb.append(yn)

            with tc.tile_pool(name="opps", bufs=4, space="PSUM") as opp, \
                 tc.tile_pool(name="obuf", bufs=2) as op_:
                for mt in range(CB):
                    for half in range(2):
                        osb = op_.tile([128, N // 2, PX], dt.float32, tag="osb", name="osb")
                        for ch4 in range(4):
                            ch = half * 4 + ch4
                            op = opp.tile([128, 512], dt.float32, tag="op", name="op")
                            for kc in range(CB):
                                nc.tensor.matmul(op[:], w2_sb[kc][:, mt * 128:(mt + 1) * 128],
                                                 yn_sb[kc][:, 2 * ch:2 * ch + 2, :],
                                                 start=(kc == 0), stop=False)
                            nc.tensor.matmul(op[:], b2_sb[0:1, mt * 128:(mt + 1) * 128], ones_r[0:1, 0:512],
                                             start=False, stop=True)
                            dst = osb[:, 2 * ch4:2 * ch4 + 2, :]
                            srcv = op[:].rearrange("c (n p) -> c n p", n=2)
                            nc.scalar.copy(dst, srcv)
                        (nc.sync if (mt + half) % 2 == 0 else nc.scalar).dma_start(out[half * 8:half * 8 + 8, mt * 128:(mt + 1) * 128, :].rearrange("n c p -> c n p"), osb[:])
            yp_cm.__exit__(None, None, None)
            xnp_cm.__exit__(None, None, None)

    nc.finalize()
    return nc





def _host_prep(inputs):
    x = np.asarray(inputs["x"], dtype=np.float32)
    w_qkv = np.asarray(inputs["w_qkv"], dtype=np.float32)
    b_qkv = np.asarray(inputs["b_qkv"], dtype=np.float32)
    w_out = np.asarray(inputs["w_out"], dtype=np.float32)
    b_out = np.asarray(inputs["b_out"], dtype=np.float32)
    asf = np.asarray(inputs["attn_scale_factor"], dtype=np.float32).reshape(HEADS)

    ln_affine = not (np.all(inputs["qnorm_w"] == 1.0) and np.all(inputs["qnorm_b"] == 0.0)
                     and np.all(inputs["knorm_w"] == 1.0) and np.all(inputs["knorm_b"] == 0.0))
    asf_key = None if np.all(asf == 1.0) else tuple(float(v) for v in asf)
    fast = (not ln_affine) and asf_key is None \
        and not np.any(b_qkv) and not np.any(b_out)

    t = np.arange(128)
    common = {
        "wq": np.ascontiguousarray(w_qkv.T).astype(bf16),
        "w2": np.ascontiguousarray(w_out.T).astype(bf16),
        "n1w": np.asarray(inputs["norm1_w"], np.float32),
        "n1b": np.asarray(inputs["norm1_b"], np.float32),
        "n2w": np.asarray(inputs["norm2_w"], np.float32),
        "n2b": np.asarray(inputs["norm2_b"], np.float32),
        "ident": np.eye(128, dtype=np.float32).astype(bf16),
    }
    mask = (t[:, None] % 8 == t[None, :] % 8).astype(np.float32)
    common["mask4"] = np.tile(mask, (1, 4)).astype(bf16)
    if fast:
        common["sel16"] = (t[:, None] // 8 == np.arange(N)[None, :]).astype(np.float32).astype(bf16)
        # rank-9 additive mask: sum_p A[p,m]*B[p,n] = -C + C*[m%8 == n%8]
        C = 240.0
        c0 = np.sqrt(C)
        A9 = np.zeros((9, 128), np.float32)
        B9 = np.zeros((9, 128), np.float32)
        A9[0, :] = c0
        B9[0, :] = -c0
        for pp in range(8):
            A9[1 + pp] = c0 * (t % 8 == pp)
            B9[1 + pp] = c0 * (t % 8 == pp)
        common["mk9"] = np.concatenate([A9, B9], axis=1).astype(bf16)
    else:
        common["bq"] = b_qkv.reshape(1, CO).astype(bf16)
        common["b2r"] = b_out.reshape(1, EMB).astype(bf16)
        if ln_affine:
            common["qgw"] = np.tile(np.asarray(inputs["qnorm_w"], np.float32), (128, 1)).astype(bf16)
            common["qgb"] = np.tile(np.asarray(inputs["qnorm_b"], np.float32), (128, 1)).astype(bf16)
            common["kgw"] = np.tile(np.asarray(inputs["knorm_w"], np.float32), (128, 1)).astype(bf16)
            common["kgb"] = np.tile(np.asarray(inputs["knorm_b"], np.float32), (128, 1)).astype(bf16)
        if asf_key is not None:
            common["bsel"] = (t[:, None] % 8 == np.arange(8)[None, :]).astype(np.float32).astype(bf16)
            common["bselT"] = (np.arange(8)[:, None] == t[None, :] % 8).astype(np.float32).astype(bf16)

    in_maps = []
    for c in range(NCORES):
        b, rb = divmod(c, 4)
        xs = np.ascontiguousarray(x[b, :, :, rb * 8:(rb + 1) * 8, :]).reshape(N, EMB, PX)
        m = dict(common)
        m["xs"] = xs
        in_maps.append(m)
    return in_maps, fast, ln_affine, asf_key


def kernel(**inputs):
    from concourse.bass_utils import run_bass_kernel_spmd

    in_maps, fast, ln_affine, asf_key = _host_prep(inputs)
    key = (fast, ln_affine, asf_key)
    if key not in _prog_cache:
        _prog_cache[key] = _build_program(fast, ln_affine, asf_key)
    nc = _prog_cache[key]
    res = run_bass_kernel_spmd(nc, in_maps, list(range(NCORES)))
    x = inputs["x"]
    full = np.empty((B, N, EMB, HH, WW), dtype=np.float32)
    for c in range(NCORES):
        b, rb = divmod(c, 4)
        full[b, :, :, rb * 8:(rb + 1) * 8, :] = res.results[c]["out"].reshape(N, EMB, 8, WW)
    return full
